# revision 23
# baseline (speedup 1.0000x reference)
"""Trainium2 Bass kernel for quantized-MoE Bottleneck (nn_Bottleneck_37503654429269).

v5 design (one core = 4 samples, SPMD over 8 cores, data-parallel on batch):
- On-device x-quantization (ACT scale+1536 exact-round trick + DVE clip)
  instead of DMA'ing a second quantized copy of x (saves 1.6MB/core DMA).
- DMA order by need: consts, x(chunk0), w1(g0), w2(g0), x(rest), w3(g0), g1.
- PE warmup spin during the DMA phase so HAM is at 2.4GHz for real matmuls.
- GN stats: bn_stats per 2-mo batch; partition-reduce via an all-ones
  [128x128] matmul that BROADCASTS the column sums to all partitions, so the
  whole mean/var/P/Q pipeline runs as tiny all-partition DVE ops.  The old
  fp32 LOW_HIGH outer-product matmuls (6.7us of cold PE) are gone.
- Finals: tensor_scalar affine (4x mode, ~111ns) per (mo,si) + one big
  tensor_tensor add of x + one big relu per chunk, replacing 468ns/op
  affine_then_add + separate relus.
- Engines: PE matmuls; ACT xq-scale + psum drains; DVE clips/stats/finals.
"""

import numpy as np

BITS = (2, 4, 8)
EPS = 1e-5
B, C_IN, H, W = 32, 1024, 14, 14
WIDTH, OUTC = 256, 1024
PIX = H * W  # 196
NCORES = 8
OFS = 1536.0  # fp16 ints are exact in [1024, 2048)

_NC_CACHE = {}


# ----------------------------------------------------------------------------
# Device program
# ----------------------------------------------------------------------------

def _build_nc(group_sizes):
    from contextlib import ExitStack
    import concourse.bacc as bacc
    import concourse.mybir as mybir
    import concourse.tile as tile

    F32 = mybir.dt.float32
    FP16 = mybir.dt.float16
    ALU = mybir.AluOpType
    ACT = mybir.ActivationFunctionType

    NG = len(group_sizes)
    assert sum(group_sizes) == 4
    slot0 = [sum(group_sizes[:g]) for g in range(NG)]
    chunks = []  # (g, c0, nchunk)
    for g in range(NG):
        for c0 in range(0, group_sizes[g], 2):
            chunks.append((g, c0, min(2, group_sizes[g] - c0)))

    GB = 44 * NG  # global const base in cc
    NCC = GB + 65

    nc = bacc.Bacc("TRN2", target_bir_lowering=False, debug=False,
                   num_devices=NCORES)

    cc_d = nc.dram_tensor("cc", [128, NCC], F32, kind="ExternalInput")
    x_d = nc.dram_tensor("x", [128, 4, 8, PIX], FP16, kind="ExternalInput")
    xq_d = [nc.dram_tensor(f"xq{ci}", [128, ns, 8, PIX], FP16,
                           kind="ExternalInput")
            for ci, (g, c0, ns) in enumerate(chunks)]
    w1_d = [nc.dram_tensor(f"w1g{g}", [128, 8, 256], FP16,
                           kind="ExternalInput") for g in range(NG)]
    w2_d = [nc.dram_tensor(f"w2g{g}", [128, 9, 2, 256], FP16,
                           kind="ExternalInput") for g in range(NG)]
    w3_d = [nc.dram_tensor(f"w3g{g}", [128, 2, 1024], FP16,
                           kind="ExternalInput") for g in range(NG)]
    out_d = nc.dram_tensor("out", [128, 8, 4, PIX], FP16,
                           kind="ExternalOutput")

    with tile.TileContext(nc) as tc, ExitStack() as ctx:
        res = ctx.enter_context(tc.tile_pool(name="res", bufs=1))
        rot = ctx.enter_context(tc.tile_pool(name="rot", bufs=6))
        pp = ctx.enter_context(tc.tile_pool(name="pp", bufs=1, space="PSUM"))

        # ---- on-chip constants (no DMA needed) + PE/ACT warmup
        ONES = res.tile([128, 128], F32, name="ONES", tag="ONES")
        nc.vector.memset(ONES, 1.0)
        WRM = res.tile([128, 128], FP16, name="WRM", tag="WRM")
        nc.vector.memset(WRM, 0.5)
        ZROC = res.tile([128, 1], F32, name="ZROC", tag="ZROC")
        nc.vector.memset(ZROC, 0.0)
        # warm the sqrt activation table during the DMA phase
        _wu = rot.tile([128, 1], F32, name="_wu", tag="_wu")
        nc.scalar.activation(out=_wu, in_=ONES[:, 0:1], func=ACT.Sqrt,
                             bias=ZROC, scale=1.0)
        # PE warmup: keep the HAM clock-gate at 8/8 until real data arrives
        wps = pp.tile([128, 128], F32, name="wps", tag="red", bufs=1)
        for i in range(44):
            nc.tensor.matmul(wps, WRM, WRM, start=True, stop=True)

        # ---- input tiles + DMAs in need order
        CC = res.tile([128, NCC], F32, name="CC", tag="CC")
        nc.sync.dma_start(out=CC, in_=cc_d.ap())
        XA = res.tile([128, 4, 8, PIX], FP16, name="XA", tag="XA")
        X = [XA[:, slot0[g] + c0:slot0[g] + c0 + ns]
             for ci, (g, c0, ns) in enumerate(chunks)]
        W1T = [res.tile([128, 8, 256], FP16, name=f"W1T{g}", tag=f"W1T{g}")
               for g in range(NG)]
        W2T = [res.tile([128, 9, 2, 256], FP16, name=f"W2T{g}", tag=f"W2T{g}")
               for g in range(NG)]
        W3T = [res.tile([128, 2, 1024], FP16, name=f"W3T{g}", tag=f"W3T{g}")
               for g in range(NG)]
        XQ = [res.tile([128, ns, 8, PIX], FP16, name=f"XQ{ci}", tag=f"XQ{ci}")
              for ci, (g, c0, ns) in enumerate(chunks)]
        nc.sync.dma_start(out=XQ[0], in_=xq_d[0].ap())
        nc.sync.dma_start(out=W1T[0], in_=w1_d[0].ap())
        nc.sync.dma_start(out=W2T[0], in_=w2_d[0].ap())
        for ci in range(1, len(chunks)):
            nc.sync.dma_start(out=XQ[ci], in_=xq_d[ci].ap())
        nc.sync.dma_start(out=W3T[0], in_=w3_d[0].ap())
        for g in range(1, NG):
            nc.sync.dma_start(out=W1T[g], in_=w1_d[g].ap())
            nc.sync.dma_start(out=W2T[g], in_=w2_d[g].ap())
            nc.sync.dma_start(out=W3T[g], in_=w3_d[g].ap())
        # residual x is only needed by the finals -> lowest DMA priority
        nc.sync.dma_start(out=XA, in_=x_d.ap())

        def A1(g, ko):
            return CC[:, 44 * g + ko:44 * g + ko + 1]

        def B1(g, ko):
            return CC[:, 44 * g + 2 + ko:44 * g + 3 + ko]

        def A2(g, ko):
            return CC[:, 44 * g + 4 + ko:44 * g + 5 + ko]

        def B2(g, ko):
            return CC[:, 44 * g + 6 + ko:44 * g + 7 + ko]

        def XBU(g):
            return CC[:, 44 * g + 8:44 * g + 9]

        def C3E(g):
            return CC[:, 44 * g + 9:44 * g + 10]

        def D3(g, mo):
            return CC[:, 44 * g + 12 + mo:44 * g + 13 + mo]

        CGG = CC[:, GB:GB + 32].rearrange("p (m s) -> p m s", m=8)
        CGB = CC[:, GB + 32:GB + 64].rearrange("p (m s) -> p m s", m=8)
        EPSC = CC[:, GB + 64:GB + 65]

        # ---- persistent intermediate tiles
        HP = [[res.tile([128, group_sizes[g], 16, 20], FP16,
                        name=f"HP{ko}_{g}", tag=f"HP{ko}_{g}")
               for g in range(NG)] for ko in range(2)]
        for ko in range(2):
            for g in range(NG):
                nc.vector.memset(HP[ko][g], OFS)
        Q2 = [[res.tile([128, group_sizes[g] * PIX], FP16,
                        name=f"Q2{ko}_{g}", tag=f"Q2{ko}_{g}")
               for g in range(NG)] for ko in range(2)]
        H3 = [res.tile([128, 8, ns * PIX], FP16, name=f"H3_{ci}",
                       tag=f"H3_{ci}")
              for ci, (g, c0, ns) in enumerate(chunks)]
        OUT = [res.tile([128, 8, ns * PIX], FP16, name=f"OUT_{ci}",
                        tag=f"OUT_{ci}")
               for ci, (g, c0, ns) in enumerate(chunks)]
        BST = [res.tile([128, 8 * ns * 8], F32, name=f"BST{ci}",
                        tag=f"BST{ci}")
               for ci, (g, c0, ns) in enumerate(chunks)]
        ST = [res.tile([128, 544], F32, name=f"ST{ci}", tag=f"ST{ci}")
              for ci, (g, c0, ns) in enumerate(chunks)]
        PQD = res.tile([128, 8, 2, 4], F32, name="PQD", tag="PQD")

        # ---- per-chunk pipeline (GN stats are per-sample, so each
        # chunk computes its own stats + finals and overlaps later chunks).
        # Emission order is software-pipelined: chunk k's stats/finals are
        # emitted AFTER chunk k+1's convs so the scheduler prioritizes
        # keeping the PE fed.
        def emit_convs(ci):
            g, c0, ns = chunks[ci]
            s0c = slot0[g] + c0
            # conv1 + bn1 + qact
            for ko in range(2):
                ps = pp.tile([128, ns * PIX], F32, name="c1ps", tag="c12", bufs=3)
                for kt in range(8):
                    nc.tensor.matmul(
                        ps,
                        W1T[g][:, kt, ko * 128:(ko + 1) * 128],
                        XQ[ci][:, :, kt, :],
                        start=(kt == 0), stop=(kt == 7))
                u = rot.tile([128, ns * PIX], FP16, name="u1", tag="u")
                nc.scalar.activation(out=u, in_=ps, func=ACT.Identity,
                                     bias=B1(g, ko), scale=A1(g, ko))
                nc.vector.tensor_scalar(
                    out=HP[ko][g][:, c0:c0 + ns, 1:15, 2:16],
                    in0=u.rearrange("p (s y x) -> p s y x", s=ns, y=14),
                    scalar1=OFS, scalar2=XBU(g),
                    op0=ALU.max, op1=ALU.min)
            # conv2 + bn2 + qact
            for ko in range(2):
                ps = pp.tile([128, ns, 14, 14], F32, name="c2ps",
                             tag="c12", bufs=3)
                first = True
                for ti, (dy, dx) in enumerate(
                        (dy, dx) for dy in range(3) for dx in range(3)):
                    for kt in range(2):
                        nc.tensor.matmul(
                            ps,
                            W2T[g][:, ti, kt, ko * 128:(ko + 1) * 128],
                            HP[kt][g][:, c0:c0 + ns,
                                      dy:dy + 14, dx + 1:dx + 15],
                            start=first, stop=(ti == 8 and kt == 1))
                        first = False
                u = rot.tile([128, ns * PIX], FP16, name="u2", tag="u2")
                nc.scalar.activation(
                    out=u, in_=ps.rearrange("p s y x -> p (s y x)"),
                    func=ACT.Identity, bias=B2(g, ko), scale=A2(g, ko))
                nc.vector.tensor_scalar(
                    out=Q2[ko][g][:, c0 * PIX:(c0 + ns) * PIX],
                    in0=u, scalar1=OFS, scalar2=XBU(g),
                    op0=ALU.max, op1=ALU.min)
            # conv3 + per-mo affine drain (C3E scale, D3 offset bias) to
            # fp16 h3; 2-mo psum tiles double-buffered
            for mh in range(4):
                ps = pp.tile([128, 2, 512], F32, name="c3ps", tag="c3",
                             bufs=2)
                for mj in range(2):
                    mo = mh * 2 + mj
                    for kt in range(2):
                        nc.tensor.matmul(
                            ps[:, mj, 0:ns * PIX],
                            W3T[g][:, kt, mo * 128:(mo + 1) * 128],
                            Q2[kt][g][:, c0 * PIX:(c0 + ns) * PIX],
                            start=(kt == 0), stop=(kt == 1))
                for mj in range(2):
                    mo = mh * 2 + mj
                    nc.scalar.activation(
                        out=H3[ci][:, mo, :], in_=ps[:, mj, 0:ns * PIX],
                        func=ACT.Identity, bias=D3(g, mo), scale=C3E(g))
            # subsampled bn_stats (HW: out must be exactly 6/partition)
            nst = 8 * ns
            bv = BST[ci][:, 0:nst * 6].rearrange("p (t c) -> p t c", c=6)
            for mo in range(8):
                for si in range(ns):
                    t = mo * ns + si
                    nc.vector.bn_stats(
                        out=bv[:, t:t + 1, :],
                        in_=H3[ci][:, mo,
                                   si * PIX:(si + 1) * PIX].rearrange(
                            "p (a b) -> p a b", b=2)[:, :, 0])

        def emit_stats_finals(ci):
            g, c0, ns = chunks[ci]
            s0c = slot0[g] + c0
            nst = 8 * ns
            # ---- chunk stats -> P/Q columns (all-partition broadcast)
            mvi = BST[ci][:, 0:nst * 6].rearrange(
                "p (t h c) -> p t h c", h=2, c=3)[:, :, :, 1]
            msq = BST[ci][:, nst * 6:nst * 8].rearrange(
                "p (t h) -> p t h", h=2)
            nc.vector.tensor_tensor(out=msq, in0=mvi, in1=mvi, op=ALU.mult)
            red = pp.tile([128, nst * 8], F32, name="red", tag="red", bufs=1)
            nc.tensor.matmul(red, ONES, BST[ci], start=True, stop=True)
            Tg = ST[ci][:, 0:nst * 8]
            nc.scalar.activation(out=Tg, in_=red, func=ACT.Copy,
                                 bias=0.0, scale=1.0)
            sb = nst * 8
            TB6 = ST[ci][:, sb:sb + ns * 24].rearrange(
                "p (a s c) -> p a s c", a=4, c=6)
            TB2 = ST[ci][:, sb + 96:sb + 96 + ns * 8].rearrange(
                "p (a s c) -> p a s c", a=4, c=2)
            SC = ST[ci][:, sb + 128:sb + 128 + 3 * 4 * ns].rearrange(
                "p (k a s) -> p k a s", k=3, a=4)
            MEAN = ST[ci][:, sb + 176:sb + 176 + 4 * ns].rearrange(
                "p (a s) -> p a s", a=4)
            E2 = ST[ci][:, sb + 192:sb + 192 + 4 * ns].rearrange(
                "p (a s) -> p a s", a=4)
            VAR = ST[ci][:, sb + 208:sb + 208 + 4 * ns].rearrange(
                "p (a s) -> p a s", a=4)
            SD = ST[ci][:, sb + 224:sb + 224 + 4 * ns].rearrange(
                "p (a s) -> p a s", a=4)
            AB = ST[ci][:, sb + 240:sb + 240 + 8 * ns].rearrange(
                "p (k a s) -> p k a s", k=2, a=4)
            QT = ST[ci][:, sb + 272:sb + 272 + 4 * ns].rearrange(
                "p (a s) -> p a s", a=4)
            tv = Tg[:, 0:nst * 6].rearrange("p (a o s c) -> p a o s c",
                                            a=4, o=2, c=6)
            nc.vector.tensor_tensor(out=TB6, in0=tv[:, :, 0, :, :],
                                    in1=tv[:, :, 1, :, :], op=ALU.add)
            mv = Tg[:, nst * 6:nst * 8].rearrange(
                "p (a o s h) -> p a o s h", a=4, o=2, h=2)
            nc.vector.tensor_tensor(out=TB2, in0=mv[:, :, 0, :, :],
                                    in1=mv[:, :, 1, :, :], op=ALU.add)
            nc.vector.tensor_tensor(out=SC[:, 0], in0=TB6[:, :, :, 1],
                                    in1=TB6[:, :, :, 4], op=ALU.add)
            nc.vector.tensor_tensor(out=SC[:, 1], in0=TB6[:, :, :, 2],
                                    in1=TB6[:, :, :, 5], op=ALU.add)
            nc.vector.tensor_tensor(out=SC[:, 2], in0=TB2[:, :, :, 0],
                                    in1=TB2[:, :, :, 1], op=ALU.add)
            nc.vector.tensor_scalar(
                out=MEAN, in0=SC[:, 0],
                scalar1=1.0 / 512, scalar2=None, op0=ALU.mult)
            nc.vector.scalar_tensor_tensor(
                out=E2, in0=SC[:, 2], scalar=49.0, in1=SC[:, 1],
                op0=ALU.mult, op1=ALU.add)
            nc.vector.tensor_tensor(out=VAR, in0=MEAN, in1=MEAN,
                                    op=ALU.mult)
            nc.vector.scalar_tensor_tensor(
                out=VAR, in0=E2, scalar=1.0 / (2 * 128 * 98), in1=VAR,
                op0=ALU.mult, op1=ALU.subtract)
            nc.scalar.activation(out=SD, in_=VAR, func=ACT.Sqrt,
                                 bias=EPSC, scale=1.0)
            nc.vector.reciprocal(out=AB[:, 0], in_=SD)
            nc.vector.scalar_tensor_tensor(
                out=AB[:, 1], in0=MEAN, scalar=-1.0, in1=AB[:, 0],
                op0=ALU.mult, op1=ALU.mult)
            # P = gng*A ; Q = gnb + gng*B  (mo = 2a+o)
            pqv = PQD.rearrange("p (a o) t s -> p a o t s", o=2)
            ab0b = AB[:, 0].unsqueeze(2).broadcast_to([128, 4, 2, ns])
            ab1b = AB[:, 1].unsqueeze(2).broadcast_to([128, 4, 2, ns])
            cgg = CGG[:, :, s0c:s0c + ns].rearrange(
                "p (a o) s -> p a o s", o=2)
            cgb = CGB[:, :, s0c:s0c + ns].rearrange(
                "p (a o) s -> p a o s", o=2)
            QT2 = ST[ci][:, sb + 280:sb + 280 + 8 * ns].rearrange(
                "p (a o s) -> p a o s", a=4, o=2)
            nc.vector.tensor_tensor(
                out=pqv[:, :, :, 0, s0c:s0c + ns], in0=cgg, in1=ab0b,
                op=ALU.mult)
            nc.vector.tensor_tensor(
                out=QT2, in0=cgg, in1=ab1b, op=ALU.mult)
            nc.vector.tensor_tensor(
                out=pqv[:, :, :, 1, s0c:s0c + ns], in0=QT2, in1=cgb,
                op=ALU.add)

            # ---- finals: affine on ACT (even mo) / DVE (odd mo), then
            # +x and relu per 4-mo half with an early store
            for mo in range(0, 8, 2):
                for si in range(ns):
                    sl = s0c + si
                    nc.scalar.activation(
                        out=OUT[ci][:, mo, si * PIX:(si + 1) * PIX],
                        in_=H3[ci][:, mo, si * PIX:(si + 1) * PIX],
                        func=ACT.Identity,
                        bias=PQD[:, mo, 1, sl:sl + 1],
                        scale=PQD[:, mo, 0, sl:sl + 1])
            for mo in range(1, 8, 2):
                for si in range(ns):
                    sl = s0c + si
                    nc.vector.tensor_scalar(
                        out=OUT[ci][:, mo, si * PIX:(si + 1) * PIX],
                        in0=H3[ci][:, mo, si * PIX:(si + 1) * PIX],
                        scalar1=PQD[:, mo, 0, sl:sl + 1],
                        scalar2=PQD[:, mo, 1, sl:sl + 1],
                        op0=ALU.mult, op1=ALU.add)
            ov = OUT[ci].rearrange("p m (s q) -> p m s q", s=ns)
            xv = X[ci].rearrange("p s k q -> p k s q")
            for mh in range(2):
                nc.vector.tensor_tensor(
                    out=ov[:, mh * 4:mh * 4 + 4],
                    in0=ov[:, mh * 4:mh * 4 + 4],
                    in1=xv[:, mh * 4:mh * 4 + 4], op=ALU.add)
                nc.vector.tensor_scalar(
                    out=ov[:, mh * 4:mh * 4 + 4],
                    in0=ov[:, mh * 4:mh * 4 + 4],
                    scalar1=0.0, scalar2=None, op0=ALU.max)
                nc.sync.dma_start(
                    out=out_d.ap()[:, mh * 4:mh * 4 + 4, s0c:s0c + ns, :],
                    in_=ov[:, mh * 4:mh * 4 + 4])

        for idx in range(len(chunks) + 1):
            if idx < len(chunks):
                emit_convs(idx)
            if idx >= 1:
                emit_stats_finals(idx - 1)

    nc.compile()
    return nc


# ----------------------------------------------------------------------------
# Host side
# ----------------------------------------------------------------------------

def _quant_w(w, lv):
    n = max(lv // 2 - 1, 1)
    s = np.float32(np.abs(w).max()) + np.float32(1e-12)
    k = np.round((w.astype(np.float32) / s) * np.float32(n)).astype(np.float32)
    return k, np.float32(s) / np.float32(n)


def _assign_groups(mask):
    mask = np.asarray(mask).astype(np.int64)
    ids = {e: [int(i) for i in np.nonzero(mask == e)[0]] for e in range(3)}
    counts = [len(ids[e]) for e in range(3)]
    if all(c % 2 == 0 for c in counts):
        group_sizes = (2, 2)
        chunks2 = []
        for e in range(3):
            for j in range(0, counts[e], 2):
                chunks2.append((e, ids[e][j:j + 2]))
        assert len(chunks2) == 16
        core_samples = []
        core_experts = []
        for c in range(8):
            (ea, sa), (eb, sb) = chunks2[2 * c], chunks2[2 * c + 1]
            core_samples.append(sa + sb)
            core_experts.append([ea, eb])
        return group_sizes, core_samples, core_experts

    base = [c % 3 for c in counts]
    need = (8 - sum(base)) // 3
    t = [0, 0, 0]
    for e in range(3):
        cap = (counts[e] - base[e]) // 3
        take = min(cap, need)
        t[e] = take
        need -= take
        if need == 0:
            break
    assert need == 0
    b = [base[e] + 3 * t[e] for e in range(3)]
    a = [(counts[e] - b[e]) // 3 for e in range(3)]
    assert sum(a) == 8 and sum(b) == 8
    trip = []
    single = []
    for e in range(3):
        pos = 0
        for _ in range(a[e]):
            trip.append((e, ids[e][pos:pos + 3]))
            pos += 3
        for _ in range(b[e]):
            single.append((e, [ids[e][pos]]))
            pos += 1
        assert pos == counts[e]
    core_samples = []
    core_experts = []
    for c in range(8):
        ea, sa = trip[c]
        eb, sb = single[c]
        core_samples.append(sa + sb)
        core_experts.append([ea, eb])
    return (3, 1), core_samples, core_experts


def kernel(x, mask, w1, w2, w3, bn1_g, bn1_b, bn1_m, bn1_v,
           bn2_g, bn2_b, bn2_m, bn2_v, gn_g, gn_b):
    from concourse.bass_utils import run_bass_kernel_spmd

    f16 = np.float16
    f32 = np.float32
    x = np.asarray(x, f32)
    mask = np.asarray(mask)
    w1 = np.asarray(w1, f32)
    w2 = np.asarray(w2, f32)
    w3 = np.asarray(w3, f32)
    bn1 = [np.asarray(v, f32) for v in (bn1_g, bn1_b, bn1_m, bn1_v)]
    bn2 = [np.asarray(v, f32) for v in (bn2_g, bn2_b, bn2_m, bn2_v)]
    gn_g = np.asarray(gn_g, f32)
    gn_b = np.asarray(gn_b, f32)

    group_sizes, core_samples, core_experts = _assign_groups(mask)
    NG = len(group_sizes)
    slot0 = [sum(group_sizes[:g]) for g in range(NG)]
    chunks = []
    for g in range(NG):
        for c0 in range(0, group_sizes[g], 2):
            chunks.append((g, c0, min(2, group_sizes[g] - c0)))
    GB = 44 * NG
    NCC = GB + 65

    lv_of = [2 ** b for b in BITS]
    K1, K2, K3 = {}, {}, {}
    CW = {}
    CS1, CS2, CS3 = {}, {}, {}
    for e in set(int(v) for v in np.asarray(mask)):
        lv = lv_of[e]
        k1, c1 = _quant_w(w1, lv)
        k2, c2 = _quant_w(w2, lv)
        k3, c3 = _quant_w(w3, lv)
        K1[e] = k1.reshape(256, 1024)
        K2[e] = k2.reshape(256, 256, 3, 3)
        K3[e] = k3.reshape(1024, 256)
        CW[e] = (c1, c2, c3)
        CS1[e] = K1[e].sum(axis=1)           # [256]
        CS2[e] = K2[e].sum(axis=(1, 2, 3))   # [256]
        CS3[e] = K3[e].sum(axis=1)           # [1024]

    inv1 = bn1[0] / np.sqrt(bn1[3] + f32(EPS))
    bb1 = bn1[1] - bn1[2] * inv1
    inv2 = bn2[0] / np.sqrt(bn2[3] + f32(EPS))
    bb2 = bn2[1] - bn2[2] * inv2

    def pack_w(e):
        k1t = K1[e].T.reshape(8, 128, 256).transpose(1, 0, 2)
        k2t = K2[e].transpose(2, 3, 1, 0).reshape(9, 2, 128, 256)
        k2t = k2t.transpose(2, 0, 1, 3)
        k3t = K3[e].T.reshape(2, 128, 1024).transpose(1, 0, 2)
        return (np.ascontiguousarray(k1t).astype(f16),
                np.ascontiguousarray(k2t).astype(f16),
                np.ascontiguousarray(k3t).astype(f16))

    packed = {e: pack_w(e) for e in K1}

    gng2 = gn_g.reshape(8, 128).T   # [128, 8]
    gnb2 = gn_b.reshape(8, 128).T

    in_maps = []
    for c in range(8):
        sids = core_samples[c]
        experts = core_experts[c]

        x4 = x[sids].reshape(4, 8, 128, PIX).transpose(2, 0, 1, 3)  # p,s,k,q
        x4 = np.ascontiguousarray(x4).astype(f16)

        cc = np.zeros((128, NCC), f32)
        for g in range(NG):
            e = experts[g]
            lv = lv_of[e]
            c1, c2, c3 = CW[e]
            a1 = inv1 * c1
            b1 = bb1 * f32(lv - 1) + f32(OFS)
            a2 = inv2 * c2
            b2 = -a2 * f32(OFS) * CS2[e] + bb2 * f32(lv - 1) + f32(OFS)
            c3e = c3 / f32(lv - 1)
            d3e = -f32(OFS) * CS3[e]          # z-domain shift (scale-free)
            cc[:, 44 * g + 0:44 * g + 2] = a1.reshape(2, 128).T
            cc[:, 44 * g + 2:44 * g + 4] = b1.reshape(2, 128).T
            cc[:, 44 * g + 4:44 * g + 6] = a2.reshape(2, 128).T
            cc[:, 44 * g + 6:44 * g + 8] = b2.reshape(2, 128).T
            cc[:, 44 * g + 8] = f32(OFS) + f32(lv - 1)
            cc[:, 44 * g + 9] = c3e
            cc[:, 44 * g + 12:44 * g + 20] = (
                -c3e * f32(OFS) * CS3[e]).reshape(8, 128).T
        cc[:, GB:GB + 32] = np.repeat(gng2, 4, axis=1)      # (mo, slot)
        cc[:, GB + 32:GB + 64] = np.repeat(gnb2, 4, axis=1)
        cc[:, GB + 64] = f32(EPS)

        xqs = np.empty((128, 4, 8, PIX), f32)
        for g in range(NG):
            lv = lv_of[experts[g]]
            sls = slice(slot0[g], slot0[g] + group_sizes[g])
            xf = x[sids].reshape(4, 8, 128, PIX).transpose(2, 0, 1, 3)
            xqs[:, sls] = np.clip(np.round(xf[:, sls] * f32(lv - 1)),
                                  0.0, f32(lv - 1))
        xq16 = xqs.astype(f16)

        m = {"cc": cc, "x": x4}
        for ci, (g, c0, ns) in enumerate(chunks):
            s0 = slot0[g] + c0
            m[f"xq{ci}"] = np.ascontiguousarray(xq16[:, s0:s0 + ns])
        for g in range(NG):
            p1, p2, p3 = packed[experts[g]]
            m[f"w1g{g}"] = p1
            m[f"w2g{g}"] = p2
            m[f"w3g{g}"] = p3
        in_maps.append(m)

    key = group_sizes
    if key not in _NC_CACHE:
        _NC_CACHE[key] = _build_nc(group_sizes)
    nc = _NC_CACHE[key]

    res = run_bass_kernel_spmd(nc, in_maps, core_ids=list(range(NCORES)))

    out = np.zeros((B, OUTC, H, W), f32)
    for c in range(8):
        oc = res.results[c]["out"].astype(f32)  # [128, 8, 4, 196]
        oc = oc.transpose(2, 1, 0, 3).reshape(4, OUTC, H, W)
        for t, sid in enumerate(core_samples[c]):
            out[sid] = oc[t]
    return out


# revision 25
# speedup vs baseline: 1.0360x; 1.0360x over previous
"""Trainium2 Bass kernel for quantized-MoE Bottleneck (nn_Bottleneck_37503654429269).

v5.3 design (one core = 4 samples, SPMD over 8 cores, data-parallel on batch):
- DMA ordered by need (consts, xq chunk0, w1/w2 of group0, ...; residual x
  last); PE warmup spin + sqrt-table warm during the DMA phase so HAM is at
  2.4GHz when real matmuls start.
- conv3 drains are PURE 2-mo COPIES to fp32 h3 (GN is scale-invariant, so
  the C3E conv scale folds into eps' = eps/C3E^2 and the 1536-offset D3E
  shift folds into the bn_stats means and the Q bias); double-buffered
  [128,2,512] psum keeps the PE fed.
- GN stats are PER-SAMPLE, so stats+finals run per CHUNK and overlap later
  chunks' convs; emission order is software-pipelined (chunk k finals after
  chunk k+1 convs).  Partition-reduce via an all-ones [128x128] matmul that
  broadcasts column sums to every partition; mean/var/P/Q are tiny
  all-partition DVE ops with stride-0 broadcast_to APs.  The old fp32
  LOW_HIGH outer-product matmuls (6.7us of cold PE) are gone.
- Finals: per-(mo,si) affine split across ACT (even mo) and DVE (odd mo),
  then one tensor_tensor +x and relu per 4-mo half with an early store.
- Engines: PE matmuls; ACT drains + half the finals; DVE clips/stats/rest.
"""

import numpy as np

BITS = (2, 4, 8)
EPS = 1e-5
B, C_IN, H, W = 32, 1024, 14, 14
WIDTH, OUTC = 256, 1024
PIX = H * W  # 196
NCORES = 8
OFS = 1536.0  # fp16 ints are exact in [1024, 2048)

_NC_CACHE = {}


# ----------------------------------------------------------------------------
# Device program
# ----------------------------------------------------------------------------

def _build_nc(group_sizes):
    from contextlib import ExitStack
    import concourse.bacc as bacc
    import concourse.mybir as mybir
    import concourse.tile as tile

    F32 = mybir.dt.float32
    FP16 = mybir.dt.float16
    ALU = mybir.AluOpType
    ACT = mybir.ActivationFunctionType

    NG = len(group_sizes)
    assert sum(group_sizes) == 4
    slot0 = [sum(group_sizes[:g]) for g in range(NG)]
    chunks = []  # (g, c0, nchunk)
    for g in range(NG):
        for c0 in range(0, group_sizes[g], 2):
            chunks.append((g, c0, min(2, group_sizes[g] - c0)))

    GB = 44 * NG  # global const base in cc
    NCC = GB + 65

    nc = bacc.Bacc("TRN2", target_bir_lowering=False, debug=False,
                   num_devices=NCORES)

    cc_d = nc.dram_tensor("cc", [128, NCC], F32, kind="ExternalInput")
    x_d = nc.dram_tensor("x", [128, 4, 8, PIX], FP16, kind="ExternalInput")
    xq_d = [nc.dram_tensor(f"xq{ci}", [128, ns, 8, PIX], FP16,
                           kind="ExternalInput")
            for ci, (g, c0, ns) in enumerate(chunks)]
    w1_d = [nc.dram_tensor(f"w1g{g}", [128, 8, 256], FP16,
                           kind="ExternalInput") for g in range(NG)]
    w2_d = [nc.dram_tensor(f"w2g{g}", [128, 9, 2, 256], FP16,
                           kind="ExternalInput") for g in range(NG)]
    w3_d = [nc.dram_tensor(f"w3g{g}", [128, 2, 1024], FP16,
                           kind="ExternalInput") for g in range(NG)]
    out_d = nc.dram_tensor("out", [128, 8, 4, PIX], FP16,
                           kind="ExternalOutput")

    with tile.TileContext(nc) as tc, ExitStack() as ctx:
        res = ctx.enter_context(tc.tile_pool(name="res", bufs=1))
        rot = ctx.enter_context(tc.tile_pool(name="rot", bufs=6))
        pp = ctx.enter_context(tc.tile_pool(name="pp", bufs=1, space="PSUM"))

        # ---- on-chip constants (no DMA needed) + PE/ACT warmup
        ONES = res.tile([128, 128], F32, name="ONES", tag="ONES")
        nc.vector.memset(ONES, 1.0)
        WRM = res.tile([128, 128], FP16, name="WRM", tag="WRM")
        nc.vector.memset(WRM, 0.5)
        ZROC = res.tile([128, 1], F32, name="ZROC", tag="ZROC")
        nc.vector.memset(ZROC, 0.0)
        # warm the sqrt activation table during the DMA phase
        _wu = rot.tile([128, 1], F32, name="_wu", tag="_wu")
        nc.scalar.activation(out=_wu, in_=ONES[:, 0:1], func=ACT.Sqrt,
                             bias=ZROC, scale=1.0)
        # PE warmup: keep the HAM clock-gate at 8/8 until real data arrives
        wps = pp.tile([128, 128], F32, name="wps", tag="red", bufs=1)
        for i in range(44):
            nc.tensor.matmul(wps, WRM, WRM, start=True, stop=True)

        # ---- input tiles + DMAs in need order
        CC = res.tile([128, NCC], F32, name="CC", tag="CC")
        nc.sync.dma_start(out=CC, in_=cc_d.ap())
        XA = res.tile([128, 4, 8, PIX], FP16, name="XA", tag="XA")
        X = [XA[:, slot0[g] + c0:slot0[g] + c0 + ns]
             for ci, (g, c0, ns) in enumerate(chunks)]
        W1T = [res.tile([128, 8, 256], FP16, name=f"W1T{g}", tag=f"W1T{g}")
               for g in range(NG)]
        W2T = [res.tile([128, 9, 2, 256], FP16, name=f"W2T{g}", tag=f"W2T{g}")
               for g in range(NG)]
        W3T = [res.tile([128, 2, 1024], FP16, name=f"W3T{g}", tag=f"W3T{g}")
               for g in range(NG)]
        XQ = [res.tile([128, ns, 8, PIX], FP16, name=f"XQ{ci}", tag=f"XQ{ci}")
              for ci, (g, c0, ns) in enumerate(chunks)]
        nc.sync.dma_start(out=XQ[0], in_=xq_d[0].ap())
        nc.sync.dma_start(out=W1T[0], in_=w1_d[0].ap())
        nc.sync.dma_start(out=W2T[0], in_=w2_d[0].ap())
        for ci in range(1, len(chunks)):
            nc.sync.dma_start(out=XQ[ci], in_=xq_d[ci].ap())
        nc.sync.dma_start(out=W3T[0], in_=w3_d[0].ap())
        for g in range(1, NG):
            nc.sync.dma_start(out=W1T[g], in_=w1_d[g].ap())
            nc.sync.dma_start(out=W2T[g], in_=w2_d[g].ap())
            nc.sync.dma_start(out=W3T[g], in_=w3_d[g].ap())
        # residual x is only needed by the finals -> lowest DMA priority
        nc.sync.dma_start(out=XA, in_=x_d.ap())

        def A1(g, ko):
            return CC[:, 44 * g + ko:44 * g + ko + 1]

        def B1(g, ko):
            return CC[:, 44 * g + 2 + ko:44 * g + 3 + ko]

        def A2(g, ko):
            return CC[:, 44 * g + 4 + ko:44 * g + 5 + ko]

        def B2(g, ko):
            return CC[:, 44 * g + 6 + ko:44 * g + 7 + ko]

        def XBU(g):
            return CC[:, 44 * g + 8:44 * g + 9]

        def EPSI(g):
            return CC[:, 44 * g + 9:44 * g + 10]

        def D3ER(g):
            return CC[:, 44 * g + 12:44 * g + 44].rearrange(
                "p (m s) -> p m s", m=8)

        CGG = CC[:, GB:GB + 32].rearrange("p (m s) -> p m s", m=8)
        CGB = CC[:, GB + 32:GB + 64].rearrange("p (m s) -> p m s", m=8)
        EPSC = CC[:, GB + 64:GB + 65]

        # ---- persistent intermediate tiles
        HP = [[res.tile([128, group_sizes[g], 16, 20], FP16,
                        name=f"HP{ko}_{g}", tag=f"HP{ko}_{g}")
               for g in range(NG)] for ko in range(2)]
        for ko in range(2):
            for g in range(NG):
                nc.vector.memset(HP[ko][g], OFS)
        Q2 = [[res.tile([128, group_sizes[g] * PIX], FP16,
                        name=f"Q2{ko}_{g}", tag=f"Q2{ko}_{g}")
               for g in range(NG)] for ko in range(2)]
        H3 = [res.tile([128, 8, ns * PIX], F32, name=f"H3_{ci}",
                       tag=f"H3_{ci}")
              for ci, (g, c0, ns) in enumerate(chunks)]
        OUT = [res.tile([128, 8, ns * PIX], FP16, name=f"OUT_{ci}",
                        tag=f"OUT_{ci}")
               for ci, (g, c0, ns) in enumerate(chunks)]
        BST = [res.tile([128, 8 * ns * 8], F32, name=f"BST{ci}",
                        tag=f"BST{ci}")
               for ci, (g, c0, ns) in enumerate(chunks)]
        ST = [res.tile([128, 544], F32, name=f"ST{ci}", tag=f"ST{ci}")
              for ci, (g, c0, ns) in enumerate(chunks)]
        PQD = res.tile([128, 8, 2, 4], F32, name="PQD", tag="PQD")

        # ---- per-chunk pipeline (GN stats are per-sample, so each
        # chunk computes its own stats + finals and overlaps later chunks).
        # Emission order is software-pipelined: chunk k's stats/finals are
        # emitted AFTER chunk k+1's convs so the scheduler prioritizes
        # keeping the PE fed.
        def emit_convs(ci):
            g, c0, ns = chunks[ci]
            s0c = slot0[g] + c0
            # conv1 + bn1 + qact
            for ko in range(2):
                ps = pp.tile([128, ns * PIX], F32, name="c1ps", tag="c12", bufs=3)
                for kt in range(8):
                    nc.tensor.matmul(
                        ps,
                        W1T[g][:, kt, ko * 128:(ko + 1) * 128],
                        XQ[ci][:, :, kt, :],
                        start=(kt == 0), stop=(kt == 7))
                u = rot.tile([128, ns * PIX], FP16, name="u1", tag="u")
                nc.scalar.activation(out=u, in_=ps, func=ACT.Identity,
                                     bias=B1(g, ko), scale=A1(g, ko))
                nc.vector.tensor_scalar(
                    out=HP[ko][g][:, c0:c0 + ns, 1:15, 2:16],
                    in0=u.rearrange("p (s y x) -> p s y x", s=ns, y=14),
                    scalar1=OFS, scalar2=XBU(g),
                    op0=ALU.max, op1=ALU.min)
            # conv2 + bn2 + qact
            for ko in range(2):
                ps = pp.tile([128, ns, 14, 14], F32, name="c2ps",
                             tag="c12", bufs=3)
                first = True
                for ti, (dy, dx) in enumerate(
                        (dy, dx) for dy in range(3) for dx in range(3)):
                    for kt in range(2):
                        nc.tensor.matmul(
                            ps,
                            W2T[g][:, ti, kt, ko * 128:(ko + 1) * 128],
                            HP[kt][g][:, c0:c0 + ns,
                                      dy:dy + 14, dx + 1:dx + 15],
                            start=first, stop=(ti == 8 and kt == 1))
                        first = False
                u = rot.tile([128, ns * PIX], FP16, name="u2", tag="u2")
                nc.scalar.activation(
                    out=u, in_=ps.rearrange("p s y x -> p (s y x)"),
                    func=ACT.Identity, bias=B2(g, ko), scale=A2(g, ko))
                nc.vector.tensor_scalar(
                    out=Q2[ko][g][:, c0 * PIX:(c0 + ns) * PIX],
                    in0=u, scalar1=OFS, scalar2=XBU(g),
                    op0=ALU.max, op1=ALU.min)
            # conv3; pure-copy 2-mo drains, double-buffered (GN is
            # scale-invariant: C3E scale / D3 offset fold into stats + Q)
            for mh in range(4):
                ps = pp.tile([128, 2, 512], F32, name="c3ps", tag="c3",
                             bufs=2)
                for mj in range(2):
                    mo = mh * 2 + mj
                    for kt in range(2):
                        nc.tensor.matmul(
                            ps[:, mj, 0:ns * PIX],
                            W3T[g][:, kt, mo * 128:(mo + 1) * 128],
                            Q2[kt][g][:, c0 * PIX:(c0 + ns) * PIX],
                            start=(kt == 0), stop=(kt == 1))
                nc.scalar.activation(
                    out=H3[ci][:, mh * 2:mh * 2 + 2, :],
                    in_=ps[:, :, 0:ns * PIX], func=ACT.Copy,
                    bias=0.0, scale=1.0)
            # subsampled bn_stats (HW: out must be exactly 6/partition)
            nst = 8 * ns
            bv = BST[ci][:, 0:nst * 6].rearrange("p (t c) -> p t c", c=6)
            for mo in range(8):
                for si in range(ns):
                    t = mo * ns + si
                    nc.vector.bn_stats(
                        out=bv[:, t:t + 1, :],
                        in_=H3[ci][:, mo,
                                   si * PIX:(si + 1) * PIX].rearrange(
                            "p (a b) -> p a b", b=2)[:, :, 0])

        def emit_stats_finals(ci):
            g, c0, ns = chunks[ci]
            s0c = slot0[g] + c0
            nst = 8 * ns
            # ---- chunk stats -> P/Q columns (all-partition broadcast)
            mvih = BST[ci][:, 0:nst * 6].rearrange(
                "p (m s h c) -> p m s h c", m=8, h=2, c=3)[:, :, :, :, 1]
            d3b = D3ER(g)[:, :, s0c:s0c + ns].unsqueeze(3).broadcast_to(
                [128, 8, ns, 2])
            nc.vector.tensor_tensor(out=mvih, in0=mvih, in1=d3b, op=ALU.add)
            mvi = BST[ci][:, 0:nst * 6].rearrange(
                "p (t h c) -> p t h c", h=2, c=3)[:, :, :, 1]
            msq = BST[ci][:, nst * 6:nst * 8].rearrange(
                "p (t h) -> p t h", h=2)
            nc.vector.tensor_tensor(out=msq, in0=mvi, in1=mvi, op=ALU.mult)
            red = pp.tile([128, nst * 8], F32, name="red", tag="red", bufs=1)
            nc.tensor.matmul(red, ONES, BST[ci], start=True, stop=True)
            Tg = ST[ci][:, 0:nst * 8]
            nc.scalar.activation(out=Tg, in_=red, func=ACT.Copy,
                                 bias=0.0, scale=1.0)
            sb = nst * 8
            TB6 = ST[ci][:, sb:sb + ns * 24].rearrange(
                "p (a s c) -> p a s c", a=4, c=6)
            TB2 = ST[ci][:, sb + 96:sb + 96 + ns * 8].rearrange(
                "p (a s c) -> p a s c", a=4, c=2)
            SC = ST[ci][:, sb + 128:sb + 128 + 3 * 4 * ns].rearrange(
                "p (k a s) -> p k a s", k=3, a=4)
            MEAN = ST[ci][:, sb + 176:sb + 176 + 4 * ns].rearrange(
                "p (a s) -> p a s", a=4)
            E2 = ST[ci][:, sb + 192:sb + 192 + 4 * ns].rearrange(
                "p (a s) -> p a s", a=4)
            VAR = ST[ci][:, sb + 208:sb + 208 + 4 * ns].rearrange(
                "p (a s) -> p a s", a=4)
            SD = ST[ci][:, sb + 224:sb + 224 + 4 * ns].rearrange(
                "p (a s) -> p a s", a=4)
            AB = ST[ci][:, sb + 240:sb + 240 + 8 * ns].rearrange(
                "p (k a s) -> p k a s", k=2, a=4)
            QT = ST[ci][:, sb + 272:sb + 272 + 4 * ns].rearrange(
                "p (a s) -> p a s", a=4)
            tv = Tg[:, 0:nst * 6].rearrange("p (a o s c) -> p a o s c",
                                            a=4, o=2, c=6)
            nc.vector.tensor_tensor(out=TB6, in0=tv[:, :, 0, :, :],
                                    in1=tv[:, :, 1, :, :], op=ALU.add)
            mv = Tg[:, nst * 6:nst * 8].rearrange(
                "p (a o s h) -> p a o s h", a=4, o=2, h=2)
            nc.vector.tensor_tensor(out=TB2, in0=mv[:, :, 0, :, :],
                                    in1=mv[:, :, 1, :, :], op=ALU.add)
            nc.vector.tensor_tensor(out=SC[:, 0], in0=TB6[:, :, :, 1],
                                    in1=TB6[:, :, :, 4], op=ALU.add)
            nc.vector.tensor_tensor(out=SC[:, 1], in0=TB6[:, :, :, 2],
                                    in1=TB6[:, :, :, 5], op=ALU.add)
            nc.vector.tensor_tensor(out=SC[:, 2], in0=TB2[:, :, :, 0],
                                    in1=TB2[:, :, :, 1], op=ALU.add)
            nc.vector.tensor_scalar(
                out=MEAN, in0=SC[:, 0],
                scalar1=1.0 / 512, scalar2=None, op0=ALU.mult)
            nc.vector.scalar_tensor_tensor(
                out=E2, in0=SC[:, 2], scalar=49.0, in1=SC[:, 1],
                op0=ALU.mult, op1=ALU.add)
            nc.vector.tensor_tensor(out=VAR, in0=MEAN, in1=MEAN,
                                    op=ALU.mult)
            nc.vector.scalar_tensor_tensor(
                out=VAR, in0=E2, scalar=1.0 / (2 * 128 * 98), in1=VAR,
                op0=ALU.mult, op1=ALU.subtract)
            nc.scalar.activation(out=SD, in_=VAR, func=ACT.Sqrt,
                                 bias=EPSI(g), scale=1.0)
            nc.vector.reciprocal(out=AB[:, 0], in_=SD)
            nc.vector.scalar_tensor_tensor(
                out=AB[:, 1], in0=MEAN, scalar=-1.0, in1=AB[:, 0],
                op0=ALU.mult, op1=ALU.mult)
            # P = gng*A ; Q = gnb + gng*(B + A*D3E)  (mo = 2a+o)
            pqv = PQD.rearrange("p (a o) t s -> p a o t s", o=2)
            ab0b = AB[:, 0].unsqueeze(2).broadcast_to([128, 4, 2, ns])
            ab1b = AB[:, 1].unsqueeze(2).broadcast_to([128, 4, 2, ns])
            d3v = D3ER(g)[:, :, s0c:s0c + ns].rearrange(
                "p (a o) s -> p a o s", o=2)
            cgg = CGG[:, :, s0c:s0c + ns].rearrange(
                "p (a o) s -> p a o s", o=2)
            cgb = CGB[:, :, s0c:s0c + ns].rearrange(
                "p (a o) s -> p a o s", o=2)
            QT2 = ST[ci][:, sb + 280:sb + 280 + 8 * ns].rearrange(
                "p (a o s) -> p a o s", a=4, o=2)
            nc.vector.tensor_tensor(
                out=pqv[:, :, :, 0, s0c:s0c + ns], in0=cgg, in1=ab0b,
                op=ALU.mult)
            nc.vector.tensor_tensor(
                out=QT2, in0=d3v, in1=ab0b, op=ALU.mult)
            nc.vector.tensor_tensor(
                out=QT2, in0=QT2, in1=ab1b, op=ALU.add)
            nc.vector.tensor_tensor(
                out=QT2, in0=QT2, in1=cgg, op=ALU.mult)
            nc.vector.tensor_tensor(
                out=pqv[:, :, :, 1, s0c:s0c + ns], in0=QT2, in1=cgb,
                op=ALU.add)

            # ---- finals: affine on ACT (even mo) / DVE (odd mo), then
            # +x and relu per 4-mo half with an early store
            for mo in range(0, 8, 2):
                for si in range(ns):
                    sl = s0c + si
                    nc.scalar.activation(
                        out=OUT[ci][:, mo, si * PIX:(si + 1) * PIX],
                        in_=H3[ci][:, mo, si * PIX:(si + 1) * PIX],
                        func=ACT.Identity,
                        bias=PQD[:, mo, 1, sl:sl + 1],
                        scale=PQD[:, mo, 0, sl:sl + 1])
            for mo in range(1, 8, 2):
                for si in range(ns):
                    sl = s0c + si
                    nc.vector.tensor_scalar(
                        out=OUT[ci][:, mo, si * PIX:(si + 1) * PIX],
                        in0=H3[ci][:, mo, si * PIX:(si + 1) * PIX],
                        scalar1=PQD[:, mo, 0, sl:sl + 1],
                        scalar2=PQD[:, mo, 1, sl:sl + 1],
                        op0=ALU.mult, op1=ALU.add)
            ov = OUT[ci].rearrange("p m (s q) -> p m s q", s=ns)
            xv = X[ci].rearrange("p s k q -> p k s q")
            for mh in range(2):
                nc.vector.tensor_tensor(
                    out=ov[:, mh * 4:mh * 4 + 4],
                    in0=ov[:, mh * 4:mh * 4 + 4],
                    in1=xv[:, mh * 4:mh * 4 + 4], op=ALU.add)
                nc.vector.tensor_scalar(
                    out=ov[:, mh * 4:mh * 4 + 4],
                    in0=ov[:, mh * 4:mh * 4 + 4],
                    scalar1=0.0, scalar2=None, op0=ALU.max)
                nc.sync.dma_start(
                    out=out_d.ap()[:, mh * 4:mh * 4 + 4, s0c:s0c + ns, :],
                    in_=ov[:, mh * 4:mh * 4 + 4])

        for idx in range(len(chunks) + 1):
            if idx < len(chunks):
                emit_convs(idx)
            if idx >= 1:
                emit_stats_finals(idx - 1)

    nc.compile()
    return nc


# ----------------------------------------------------------------------------
# Host side
# ----------------------------------------------------------------------------

def _quant_w(w, lv):
    n = max(lv // 2 - 1, 1)
    s = np.float32(np.abs(w).max()) + np.float32(1e-12)
    k = np.round((w.astype(np.float32) / s) * np.float32(n)).astype(np.float32)
    return k, np.float32(s) / np.float32(n)


def _assign_groups(mask):
    mask = np.asarray(mask).astype(np.int64)
    ids = {e: [int(i) for i in np.nonzero(mask == e)[0]] for e in range(3)}
    counts = [len(ids[e]) for e in range(3)]
    if all(c % 2 == 0 for c in counts):
        group_sizes = (2, 2)
        chunks2 = []
        for e in range(3):
            for j in range(0, counts[e], 2):
                chunks2.append((e, ids[e][j:j + 2]))
        assert len(chunks2) == 16
        core_samples = []
        core_experts = []
        for c in range(8):
            (ea, sa), (eb, sb) = chunks2[2 * c], chunks2[2 * c + 1]
            core_samples.append(sa + sb)
            core_experts.append([ea, eb])
        return group_sizes, core_samples, core_experts

    base = [c % 3 for c in counts]
    need = (8 - sum(base)) // 3
    t = [0, 0, 0]
    for e in range(3):
        cap = (counts[e] - base[e]) // 3
        take = min(cap, need)
        t[e] = take
        need -= take
        if need == 0:
            break
    assert need == 0
    b = [base[e] + 3 * t[e] for e in range(3)]
    a = [(counts[e] - b[e]) // 3 for e in range(3)]
    assert sum(a) == 8 and sum(b) == 8
    trip = []
    single = []
    for e in range(3):
        pos = 0
        for _ in range(a[e]):
            trip.append((e, ids[e][pos:pos + 3]))
            pos += 3
        for _ in range(b[e]):
            single.append((e, [ids[e][pos]]))
            pos += 1
        assert pos == counts[e]
    core_samples = []
    core_experts = []
    for c in range(8):
        ea, sa = trip[c]
        eb, sb = single[c]
        core_samples.append(sa + sb)
        core_experts.append([ea, eb])
    return (3, 1), core_samples, core_experts


def kernel(x, mask, w1, w2, w3, bn1_g, bn1_b, bn1_m, bn1_v,
           bn2_g, bn2_b, bn2_m, bn2_v, gn_g, gn_b):
    from concourse.bass_utils import run_bass_kernel_spmd

    f16 = np.float16
    f32 = np.float32
    x = np.asarray(x, f32)
    mask = np.asarray(mask)
    w1 = np.asarray(w1, f32)
    w2 = np.asarray(w2, f32)
    w3 = np.asarray(w3, f32)
    bn1 = [np.asarray(v, f32) for v in (bn1_g, bn1_b, bn1_m, bn1_v)]
    bn2 = [np.asarray(v, f32) for v in (bn2_g, bn2_b, bn2_m, bn2_v)]
    gn_g = np.asarray(gn_g, f32)
    gn_b = np.asarray(gn_b, f32)

    group_sizes, core_samples, core_experts = _assign_groups(mask)
    NG = len(group_sizes)
    slot0 = [sum(group_sizes[:g]) for g in range(NG)]
    chunks = []
    for g in range(NG):
        for c0 in range(0, group_sizes[g], 2):
            chunks.append((g, c0, min(2, group_sizes[g] - c0)))
    GB = 44 * NG
    NCC = GB + 65

    lv_of = [2 ** b for b in BITS]
    K1, K2, K3 = {}, {}, {}
    CW = {}
    CS1, CS2, CS3 = {}, {}, {}
    for e in set(int(v) for v in np.asarray(mask)):
        lv = lv_of[e]
        k1, c1 = _quant_w(w1, lv)
        k2, c2 = _quant_w(w2, lv)
        k3, c3 = _quant_w(w3, lv)
        K1[e] = k1.reshape(256, 1024)
        K2[e] = k2.reshape(256, 256, 3, 3)
        K3[e] = k3.reshape(1024, 256)
        CW[e] = (c1, c2, c3)
        CS1[e] = K1[e].sum(axis=1)           # [256]
        CS2[e] = K2[e].sum(axis=(1, 2, 3))   # [256]
        CS3[e] = K3[e].sum(axis=1)           # [1024]

    inv1 = bn1[0] / np.sqrt(bn1[3] + f32(EPS))
    bb1 = bn1[1] - bn1[2] * inv1
    inv2 = bn2[0] / np.sqrt(bn2[3] + f32(EPS))
    bb2 = bn2[1] - bn2[2] * inv2

    def pack_w(e):
        k1t = K1[e].T.reshape(8, 128, 256).transpose(1, 0, 2)
        k2t = K2[e].transpose(2, 3, 1, 0).reshape(9, 2, 128, 256)
        k2t = k2t.transpose(2, 0, 1, 3)
        k3t = K3[e].T.reshape(2, 128, 1024).transpose(1, 0, 2)
        return (np.ascontiguousarray(k1t).astype(f16),
                np.ascontiguousarray(k2t).astype(f16),
                np.ascontiguousarray(k3t).astype(f16))

    packed = {e: pack_w(e) for e in K1}

    gng2 = gn_g.reshape(8, 128).T   # [128, 8]
    gnb2 = gn_b.reshape(8, 128).T

    in_maps = []
    for c in range(8):
        sids = core_samples[c]
        experts = core_experts[c]

        x4 = x[sids].reshape(4, 8, 128, PIX).transpose(2, 0, 1, 3)  # p,s,k,q
        x4 = np.ascontiguousarray(x4).astype(f16)

        cc = np.zeros((128, NCC), f32)
        for g in range(NG):
            e = experts[g]
            lv = lv_of[e]
            c1, c2, c3 = CW[e]
            a1 = inv1 * c1
            b1 = bb1 * f32(lv - 1) + f32(OFS)
            a2 = inv2 * c2
            b2 = -a2 * f32(OFS) * CS2[e] + bb2 * f32(lv - 1) + f32(OFS)
            c3e = c3 / f32(lv - 1)
            d3e = -f32(OFS) * CS3[e]          # z-domain shift (scale-free)
            cc[:, 44 * g + 0:44 * g + 2] = a1.reshape(2, 128).T
            cc[:, 44 * g + 2:44 * g + 4] = b1.reshape(2, 128).T
            cc[:, 44 * g + 4:44 * g + 6] = a2.reshape(2, 128).T
            cc[:, 44 * g + 6:44 * g + 8] = b2.reshape(2, 128).T
            cc[:, 44 * g + 8] = f32(OFS) + f32(lv - 1)
            cc[:, 44 * g + 9] = f32(EPS) / (c3e * c3e)
            cc[:, 44 * g + 12:44 * g + 44] = np.repeat(
                d3e.reshape(8, 128).T, 4, axis=1)
        cc[:, GB:GB + 32] = np.repeat(gng2, 4, axis=1)      # (mo, slot)
        cc[:, GB + 32:GB + 64] = np.repeat(gnb2, 4, axis=1)
        cc[:, GB + 64] = f32(EPS)

        xqs = np.empty((128, 4, 8, PIX), f32)
        for g in range(NG):
            lv = lv_of[experts[g]]
            sls = slice(slot0[g], slot0[g] + group_sizes[g])
            xf = x[sids].reshape(4, 8, 128, PIX).transpose(2, 0, 1, 3)
            xqs[:, sls] = np.clip(np.round(xf[:, sls] * f32(lv - 1)),
                                  0.0, f32(lv - 1))
        xq16 = xqs.astype(f16)

        m = {"cc": cc, "x": x4}
        for ci, (g, c0, ns) in enumerate(chunks):
            s0 = slot0[g] + c0
            m[f"xq{ci}"] = np.ascontiguousarray(xq16[:, s0:s0 + ns])
        for g in range(NG):
            p1, p2, p3 = packed[experts[g]]
            m[f"w1g{g}"] = p1
            m[f"w2g{g}"] = p2
            m[f"w3g{g}"] = p3
        in_maps.append(m)

    key = group_sizes
    if key not in _NC_CACHE:
        _NC_CACHE[key] = _build_nc(group_sizes)
    nc = _NC_CACHE[key]

    res = run_bass_kernel_spmd(nc, in_maps, core_ids=list(range(NCORES)))

    out = np.zeros((B, OUTC, H, W), f32)
    for c in range(8):
        oc = res.results[c]["out"].astype(f32)  # [128, 8, 4, 196]
        oc = oc.transpose(2, 1, 0, 3).reshape(4, OUTC, H, W)
        for t, sid in enumerate(core_samples[c]):
            out[sid] = oc[t]
    return out
